# revision 3
# baseline (speedup 1.0000x reference)
"""Trainium2 Bass kernel for nn_LSTMPredictor (autoregressive LSTM decoder).

Contract: kernel(**inputs) takes FULL unsharded inputs (as produced by
reference.setup_inputs) and returns the FULL (8192, 32, 12) float32 output.

Strategy
--------
Pure data parallel: batch 8192 is split across 8 NeuronCores (1024 each),
weights replicated. All state is kept TRANSPOSED on-chip: h^T/c^T are
[H rows on partitions, batch on free dim], so the recurrent matmul
gates^T = W @ h^T needs no per-step transposes.

Algebraic fusion: the autoregressive input x_t = pred_{t-1} = h_t @ Wout.T
+ bout uses the *same* h_t as the recurrent matmul of step t, so
    gates_t = h_t @ (Whh + Wih@Wout).T + (bih + bhh + Wih@bout)   (t >= 1)
    gates_0 = h_0 @ Whh.T + (bih + bhh + Wih@start_token)
which removes the input-side matmul entirely. The per-step pred matmul
(h_{t+1} @ Wout.T) is kept on-device; bout is added on the host during
the gather (it is a per-element constant).

Per step (per core): 16 matmuls N=512 (gates) + 4 (pred) on PE; 8 gate
activations + 2 tanh(c) on ACT (bias fused via the activation bias port,
reading PSUM directly); 8 tensor_tensor ops on DVE; 1 DMA out.
"""

import os

import numpy as np

import concourse.bacc as bacc
import concourse.bass as bass  # noqa: F401  (engine namespaces live on nc)
import concourse.mybir as mybir
import concourse.tile as tile
from concourse.bass_utils import run_bass_kernel_spmd

N_CORES = 8
B = 8192
BC = B // N_CORES  # 1024 batch per core
H = 256            # LSTM hidden
G4 = 4 * H         # 1024 gate rows
E = 12             # event dim
C = 128            # ctx hidden
T = 32             # steps
DI = 18            # obs(16) + act(2)
NCH = BC // 512    # 512-wide N chunks per core (2)

F32 = mybir.dt.float32
SIG = mybir.ActivationFunctionType.Sigmoid
TANH = mybir.ActivationFunctionType.Tanh
RELU = mybir.ActivationFunctionType.Relu

_CACHED_NC = None


def _build_nc():
    nc = bacc.Bacc(None, target_bir_lowering=True)

    xT_d = nc.dram_tensor("xT", [DI, BC], F32, kind="ExternalInput")
    wctxT_d = nc.dram_tensor("wctxT", [DI, C], F32, kind="ExternalInput")
    wh0T_d = nc.dram_tensor("wh0T", [C, H], F32, kind="ExternalInput")
    wc0T_d = nc.dram_tensor("wc0T", [C, H], F32, kind="ExternalInput")
    whhT_d = nc.dram_tensor("whhT", [H, G4], F32, kind="ExternalInput")
    weffT_d = nc.dram_tensor("weffT", [H, G4], F32, kind="ExternalInput")
    woutT_d = nc.dram_tensor("woutT", [H, E], F32, kind="ExternalInput")
    bctx_d = nc.dram_tensor("bctx_col", [C, 1], F32, kind="ExternalInput")
    bh0_d = nc.dram_tensor("bh0_cols", [128, H // 128], F32, kind="ExternalInput")
    bc0_d = nc.dram_tensor("bc0_cols", [128, H // 128], F32, kind="ExternalInput")
    b0_d = nc.dram_tensor("b0_cols", [128, G4 // 128], F32, kind="ExternalInput")
    beff_d = nc.dram_tensor("beff_cols", [128, G4 // 128], F32, kind="ExternalInput")
    out_d = nc.dram_tensor("preds", [T, E, BC], F32, kind="ExternalOutput")

    with tile.TileContext(nc) as tc:
        with (
            tc.tile_pool(name="consts", bufs=1) as consts,
            tc.tile_pool(name="state", bufs=2) as state,
            tc.tile_pool(name="acts", bufs=2) as acts,
            tc.tile_pool(name="pred", bufs=2) as predp,
            tc.tile_pool(name="psg", bufs=3, space="PSUM") as psg,
            tc.tile_pool(name="psp", bufs=1, space="PSUM") as psp,
        ):
            # ---- load constants/weights into SBUF ----
            xT = consts.tile([DI, BC], F32, tag="xT")
            nc.sync.dma_start(xT[:], xT_d[:])
            wctxT = consts.tile([DI, C], F32, tag="wctxT")
            nc.sync.dma_start(wctxT[:], wctxT_d[:])
            wh0T = consts.tile([C, H], F32, tag="wh0T")
            nc.sync.dma_start(wh0T[:], wh0T_d[:])
            wc0T = consts.tile([C, H], F32, tag="wc0T")
            nc.sync.dma_start(wc0T[:], wc0T_d[:])
            whhT = [consts.tile([128, G4], F32, tag=f"whhT{k}", name=f"whhT{k}") for k in range(2)]
            weffT = [consts.tile([128, G4], F32, tag=f"weffT{k}", name=f"weffT{k}") for k in range(2)]
            woutT = [consts.tile([128, E], F32, tag=f"woutT{k}", name=f"woutT{k}") for k in range(2)]
            for k in range(2):
                nc.sync.dma_start(whhT[k][:], whhT_d[k * 128:(k + 1) * 128, :])
                nc.sync.dma_start(weffT[k][:], weffT_d[k * 128:(k + 1) * 128, :])
                nc.sync.dma_start(woutT[k][:], woutT_d[k * 128:(k + 1) * 128, :])
            bctx = consts.tile([C, 1], F32, tag="bctx")
            nc.sync.dma_start(bctx[:], bctx_d[:])
            bh0 = consts.tile([128, H // 128], F32, tag="bh0")
            nc.sync.dma_start(bh0[:], bh0_d[:])
            bc0 = consts.tile([128, H // 128], F32, tag="bc0")
            nc.sync.dma_start(bc0[:], bc0_d[:])
            b0 = consts.tile([128, G4 // 128], F32, tag="b0")
            nc.sync.dma_start(b0[:], b0_d[:])
            beff = consts.tile([128, G4 // 128], F32, tag="beff")
            nc.sync.dma_start(beff[:], beff_d[:])

            # ---- context encoder: ctx = relu(Wctx @ x^T + bctx) ----
            ctx_ps = psg.tile([128, BC], F32, tag="gates")
            for n in range(NCH):
                nc.tensor.matmul(
                    ctx_ps[:, n * 512:(n + 1) * 512],
                    wctxT[:, :],
                    xT[:, n * 512:(n + 1) * 512],
                    start=True, stop=True,
                )
            ctx_sb = acts.tile([128, BC], F32, tag="ctx")
            nc.scalar.activation(ctx_sb[:], ctx_ps[:], RELU, bias=bctx[:, 0:1])

            # ---- h0 = tanh(Wh0 @ ctx + bh0), c0 = tanh(Wc0 @ ctx + bc0) ----
            h_cur = [state.tile([128, BC], F32, tag=f"h{k}", name=f"h0_{k}") for k in range(2)]
            c_cur = [state.tile([128, BC], F32, tag=f"c{k}", name=f"c0_{k}") for k in range(2)]
            for m, (wT, bcols, dst) in enumerate(
                [(wh0T, bh0, h_cur), (wc0T, bc0, c_cur)]
            ):
                for kchunk in range(2):
                    ps = psg.tile([128, BC], F32, tag="gates")
                    for n in range(NCH):
                        nc.tensor.matmul(
                            ps[:, n * 512:(n + 1) * 512],
                            wT[:, kchunk * 128:(kchunk + 1) * 128],
                            ctx_sb[:, n * 512:(n + 1) * 512],
                            start=True, stop=True,
                        )
                    nc.scalar.activation(
                        dst[kchunk][:], ps[:], TANH,
                        bias=bcols[:, kchunk:kchunk + 1],
                    )

            # ---- 32 recurrent steps, fully unrolled ----
            for t in range(T):
                wT = whhT if t == 0 else weffT
                bcols = b0 if t == 0 else beff
                h_next = [state.tile([128, BC], F32, tag=f"h{k}", name=f"h_t{t}_{k}") for k in range(2)]
                c_next = [state.tile([128, BC], F32, tag=f"c{k}", name=f"c_t{t}_{k}") for k in range(2)]
                for j in range(2):
                    # gate order i, f, g, o ; M-tile index m = gate*2 + j
                    gps = []
                    for g in range(4):
                        m = g * 2 + j
                        ps = psg.tile([128, BC], F32, tag="gates", name=f"gps_t{t}_{j}_{g}")
                        for k in range(2):  # K outer: one weight load, 2 matmuls
                            for n in range(NCH):
                                nc.tensor.matmul(
                                    ps[:, n * 512:(n + 1) * 512],
                                    wT[k][:, m * 128:(m + 1) * 128],
                                    h_cur[k][:, n * 512:(n + 1) * 512],
                                    start=(k == 0), stop=(k == 1),
                                )
                        gps.append(ps)
                    si = acts.tile([128, BC], F32, tag=f"si{j}")
                    sf = acts.tile([128, BC], F32, tag=f"sf{j}")
                    tg = acts.tile([128, BC], F32, tag=f"tg{j}")
                    so = acts.tile([128, BC], F32, tag=f"so{j}")
                    bb = lambda g: bcols[:, g * 2 + j:g * 2 + j + 1]  # noqa: E731
                    nc.scalar.activation(si[:], gps[0][:], SIG, bias=bb(0))
                    nc.scalar.activation(sf[:], gps[1][:], SIG, bias=bb(1))
                    nc.scalar.activation(tg[:], gps[2][:], TANH, bias=bb(2))
                    nc.scalar.activation(so[:], gps[3][:], SIG, bias=bb(3))
                    p1 = acts.tile([128, BC], F32, tag=f"p1{j}")
                    p2 = acts.tile([128, BC], F32, tag=f"p2{j}")
                    nc.vector.tensor_mul(p1[:], si[:], tg[:])
                    nc.vector.tensor_mul(p2[:], sf[:], c_cur[j][:])
                    nc.vector.tensor_add(c_next[j][:], p1[:], p2[:])
                    tcj = acts.tile([128, BC], F32, tag=f"tc{j}")
                    nc.scalar.activation(tcj[:], c_next[j][:], TANH)
                    nc.vector.tensor_mul(h_next[j][:], so[:], tcj[:])

                # pred_t = h_{t+1} @ Wout.T   (bout added on host)
                pp = psp.tile([E, BC], F32, tag="pred")
                for k in range(2):
                    for n in range(NCH):
                        nc.tensor.matmul(
                            pp[:, n * 512:(n + 1) * 512],
                            woutT[k][:, :],
                            h_next[k][:, n * 512:(n + 1) * 512],
                            start=(k == 0), stop=(k == 1),
                        )
                ps_out = predp.tile([E, BC], F32, tag="predsb")
                nc.vector.tensor_copy(ps_out[:], pp[:])
                nc.sync.dma_start(out_d[t], ps_out[:])

                h_cur, c_cur = h_next, c_next

    nc.compile()
    return nc


def kernel(obs, act, Wctx, bctx, Wh0, bh0, Wc0, bc0,
           Wih, Whh, bih, bhh, Wout, bout, start_token):
    global _CACHED_NC
    f32 = np.float32
    obs = np.asarray(obs, f32)
    act = np.asarray(act, f32)
    Wctx = np.asarray(Wctx, f32)
    Wih = np.asarray(Wih, f32)
    Whh = np.asarray(Whh, f32)
    Wout = np.asarray(Wout, f32)

    # host-side weight prep (tiny O(weights) work)
    xT = np.ascontiguousarray(np.concatenate([obs, act], axis=1).T)  # (18, B)
    wctxT = np.ascontiguousarray(Wctx.T)                             # (18, 128)
    wh0T = np.ascontiguousarray(np.asarray(Wh0, f32).T)              # (128, 256)
    wc0T = np.ascontiguousarray(np.asarray(Wc0, f32).T)
    whhT = np.ascontiguousarray(Whh.T)                               # (256, 1024)
    weff = Whh + Wih @ Wout                                          # (1024, 256)
    weffT = np.ascontiguousarray(weff.T)
    woutT = np.ascontiguousarray(Wout.T)                             # (256, 12)
    bsum = np.asarray(bih, f32) + np.asarray(bhh, f32)
    b0 = bsum + Wih @ np.asarray(start_token, f32)                   # (1024,)
    beff = bsum + Wih @ np.asarray(bout, f32)
    b0_cols = np.ascontiguousarray(b0.reshape(8, 128).T)             # (128, 8)
    beff_cols = np.ascontiguousarray(beff.reshape(8, 128).T)
    bh0_cols = np.ascontiguousarray(np.asarray(bh0, f32).reshape(2, 128).T)
    bc0_cols = np.ascontiguousarray(np.asarray(bc0, f32).reshape(2, 128).T)
    bctx_col = np.ascontiguousarray(np.asarray(bctx, f32).reshape(128, 1))

    if _CACHED_NC is None:
        _CACHED_NC = _build_nc()
    nc = _CACHED_NC

    shared = {
        "wctxT": wctxT, "wh0T": wh0T, "wc0T": wc0T, "whhT": whhT,
        "weffT": weffT, "woutT": woutT, "bctx_col": bctx_col,
        "bh0_cols": bh0_cols, "bc0_cols": bc0_cols,
        "b0_cols": b0_cols, "beff_cols": beff_cols,
    }
    in_maps = []
    for c in range(N_CORES):
        m = dict(shared)
        m["xT"] = np.ascontiguousarray(xT[:, c * BC:(c + 1) * BC])
        in_maps.append(m)

    trace = os.environ.get("LSTM_KERNEL_TRACE") == "1"
    kw = {}
    if trace:
        kw["trace"] = True
        td = os.environ.get("LSTM_KERNEL_TRACE_DIR")
        if td:
            import shutil
            shutil.rmtree(td, ignore_errors=True)
            os.makedirs(td, exist_ok=True)
            kw["tmpdir"] = td
    res = run_bass_kernel_spmd(nc, in_maps, list(range(N_CORES)), **kw)
    if trace:
        print(f"HW exec time: {res.exec_time_ns} ns")
        kernel.last_exec_time_ns = res.exec_time_ns
        kernel.last_profile_json = res.profile_json
        it = res.instructions_and_trace
        kernel.last_trace_path = it[1] if it is not None else None

    out = np.empty((B, T, E), f32)
    boutf = np.asarray(bout, f32)
    for c in range(N_CORES):
        r = res.results[c]["preds"]  # (T, E, BC)
        out[c * BC:(c + 1) * BC] = np.transpose(r, (2, 0, 1))
    out += boutf[None, None, :]
    return out
